# revision 14
# baseline (speedup 1.0000x reference)
"""Causal self-attention on 8 Trainium2 NeuronCores.

Sharding: 2 heads per core (tensor parallel).  The host pre-transposes the
activations/weights into the layouts the PE array wants, each core computes
QKV -> causal attention -> its partial of the output projection for its two
heads, and the host sums the 8 partial projections (row-parallel linear).

Per-core device program (SPMD, different data per core):
  xT    [1024, 4096]  x transposed, rows=embed c, cols=token t (t = b*2048+tt)
  wqkvT [1024, 384]   w_attn rows for this core's heads, transposed.
                      f = [q_h0 d0..63 | q_h1 | k_h0 | k_h1 | v_h0 | v_h1]
  wpT   [128, 1024]   w_proj columns for this core's channels, transposed
  y     [4096, 1024]  partial output (sum over cores = final)

Dataflow (everything "transposed" so the PE contraction dim is the partition
dim with no on-device transposes of activations):
  qkvT[f, t]   = wqkvT_tile.T @ xT_tile            (accumulate over 8 c-tiles)
  S^T[kt, qt]  = kT_tile.T @ qT_block              (K = head dim 64)
  P^T          = exp(S^T / 32)                     (ACT; no max subtraction --
                                                    scores are O(1), exp safe)
  causal mask  = multiply diagonal 128x128 block by 0/1 lower-tri tile
  outT[65,qt] += [V | ones].T @ P^T                (row 64 = softmax sums)
  attnT        = outT[0:64] * (1 / outT[64])       (broadcast along partitions)
  y[t, f]      = attnT_tile.T @ wpT                (partial; host sums cores)

All matmuls run as float32r (fp32 bitcast): 1 PE cycle/row when the moving
free dim is >= 256 -- full bf16-class speed with ~fp22 mantissa precision.
"""

import numpy as np

B, T, C = 2, 2048, 1024
H, D = 16, 64
NCORES = 8
HPC = H // NCORES          # heads per core = 2
BT = B * T                 # 4096 tokens total
TB = 512                   # token block (matmul moving free dim)
CK = C // 128              # 8 contraction tiles for the projections
NTB = BT // TB             # 8 token blocks
NQB = T // TB              # 4 q blocks per batch
NKT = T // 128             # 16 kt tiles per batch
SCALE = 1.0 / 32.0         # 1 / sqrt(C)


def build_program():
    """Build the single-core Bass program (same program runs on all 8 cores)."""
    from contextlib import ExitStack

    import concourse.mybir as mybir
    import concourse.tile as tile
    from concourse import bacc, library_config

    dt = mybir.dt
    F32 = dt.float32
    F32R = dt.float32r

    nc = bacc.Bacc("TRN2")
    xT = nc.dram_tensor("xT", [C, BT], F32, kind="ExternalInput").ap()
    wqkvT = nc.dram_tensor("wqkvT", [C, 3 * HPC * D], F32, kind="ExternalInput").ap()
    wpT = nc.dram_tensor("wpT", [HPC * D, C], F32, kind="ExternalInput").ap()
    # consts[0] = 128x128 identity, consts[1] = causal keep-mask
    # (mask[kt, qt] = 1.0 where kt <= qt)
    consts = nc.dram_tensor("consts", [2, 128, 128], F32, kind="ExternalInput").ap()
    y = nc.dram_tensor("y", [BT, C], F32, kind="ExternalOutput").ap()

    with ExitStack() as ctx:
        tc = ctx.enter_context(tile.TileContext(nc))
        const = ctx.enter_context(tc.tile_pool(name="const", bufs=1))
        xpool = ctx.enter_context(tc.tile_pool(name="xload", bufs=4))
        ppool = ctx.enter_context(tc.tile_pool(name="pexp", bufs=4))
        npool = ctx.enter_context(tc.tile_pool(name="norm", bufs=2))
        ypool = ctx.enter_context(tc.tile_pool(name="yout", bufs=3))
        psA = ctx.enter_context(tc.tile_pool(name="psA", bufs=3, space="PSUM"))
        psPV = ctx.enter_context(tc.tile_pool(name="psPV", bufs=2, space="PSUM"))
        psTR = ctx.enter_context(tc.tile_pool(name="psTR", bufs=2, space="PSUM"))

        # ---------- constants / persistent SBUF ----------
        w_sb = const.tile([128, CK, 3 * HPC * D], F32R, name="w_sb")
        nc.sync.dma_start(w_sb[:], wqkvT.rearrange("(a p) f -> p a f", p=128).bitcast(F32R))
        wp_sb = const.tile([128, C], F32R, name="wp_sb")
        nc.sync.dma_start(wp_sb[:], wpT.bitcast(F32R))

        ident = const.tile([128, 128], F32R, name="ident")
        nc.sync.dma_start(ident[:], consts[0].bitcast(F32R))
        trimask = const.tile([128, 128], F32, name="trimask")
        nc.sync.dma_start(trimask[:], consts[1])
        # partition_broadcast lives in the "attn" GPSIMD library; same-engine
        # FIFO order guarantees this lands before the broadcasts.
        nc.gpsimd.load_library(library_config.attn)

        # Per-batch transposed activations, heads packed on partitions
        # (h0 -> partitions 0:64, h1 -> 64:128).
        qT = [const.tile([128, T], F32R, name=f"qT{b}") for b in range(B)]
        kT = [const.tile([128, T], F32R, name=f"kT{b}") for b in range(B)]
        vT = [const.tile([128, T], F32R, name=f"vT{b}") for b in range(B)]
        attnT = [const.tile([128, T], F32R, name=f"attnT{b}") for b in range(B)]

        # [V | ones] stationary tiles for PV: V1[:, b, h, kti, 0:64] = V natural
        # [kt, d]; column 64 = 1.0 so PV row 64 accumulates the softmax sums.
        V1 = const.tile([128, B, HPC, NKT, 65], F32R, name="V1")
        nc.vector.memset(V1[:, :, :, :, 64:65].bitcast(F32), 1.0)

        # ---------- phase 1: QKV projection ----------
        dest = {0: qT, 1: kT, 2: vT}
        for tb in range(NTB):
            b, tcol = divmod(tb, NTB // B)
            xts = []
            for ci in range(CK):
                xt = xpool.tile([128, TB], F32R, name="xt", tag="xt")
                nc.sync.dma_start(
                    xt[:],
                    xT[ci * 128 : (ci + 1) * 128, tb * TB : (tb + 1) * TB].bitcast(F32R),
                )
                xts.append(xt)
            for fi in range(3):
                ps = psA.tile([128, TB], F32, name="qkv_ps", tag="psA")
                for ci in range(CK):
                    nc.tensor.matmul(
                        ps[:],
                        w_sb[:, ci, fi * 128 : (fi + 1) * 128],
                        xts[ci][:],
                        start=(ci == 0),
                        stop=(ci == CK - 1),
                    )
                nc.vector.tensor_copy(
                    out=dest[fi][b][:, tcol * TB : (tcol + 1) * TB], in_=ps[:]
                )

            # As soon as a batch's vT is complete, build its V-natural tiles
            # (PE transpose of 64-row slices through the identity).
            if tcol == NTB // B - 1:
                for h in range(HPC):
                    hp = slice(h * 64, (h + 1) * 64)
                    for kti in range(NKT):
                        tr = psTR.tile([128, 64], F32R, name="vtr", tag="psTR")
                        nc.tensor.transpose(
                            tr[:], vT[b][hp, kti * 128 : (kti + 1) * 128], ident[hp, hp]
                        )
                        nc.vector.tensor_copy(out=V1[:, b, h, kti, 0:64], in_=tr[:])

        # ---------- phase 2: causal attention (per batch / head / q-block) ----
        for b in range(B):
            for h in range(HPC):
                hp = slice(h * 64, (h + 1) * 64)
                for qb in range(NQB):
                    pv = psPV.tile([65, TB], F32, name="pv_ps", tag="psPV")
                    nkt = 4 * qb + 4
                    for kti in range(nkt):
                        qs = max(0, kti * 128 - qb * TB)  # local col start
                        N = TB - qs
                        sps = psA.tile([128, TB], F32, name="s_ps", tag="psA")
                        nc.tensor.matmul(
                            sps[:, 0:N],
                            kT[b][hp, kti * 128 : (kti + 1) * 128],
                            qT[b][hp, qb * TB + qs : (qb + 1) * TB],
                            start=True,
                            stop=True,
                        )
                        P = ppool.tile([128, TB], F32R, name="Pt", tag="P")
                        nc.scalar.activation(
                            P[:, 0:N],
                            sps[:, 0:N],
                            mybir.ActivationFunctionType.Exp,
                            scale=SCALE,
                        )
                        if kti * 128 >= qb * TB:
                            # diagonal tile: first 128 cols hold the triangle
                            nc.vector.tensor_mul(P[:, 0:128], P[:, 0:128], trimask[:])
                        nc.tensor.matmul(
                            pv[:, qs:TB],
                            V1[:, b, h, kti, :],
                            P[:, 0:N],
                            start=(kti == 0),
                            stop=(kti == nkt - 1),
                        )
                    # normalize: rows 0:64 / row 64, tokens on the free dim.
                    # partition_broadcast requires its source at partition 0.
                    rt = npool.tile([1, TB], F32, name="rt", tag="rt")
                    nc.vector.reciprocal(rt[:], pv[64:65, :])
                    bc = npool.tile([64, TB], F32, name="bc", tag="bc")
                    nc.gpsimd.partition_broadcast(bc[:], rt[:])
                    nc.vector.tensor_mul(
                        attnT[b][hp, qb * TB : (qb + 1) * TB], pv[0:64, :], bc[:]
                    )

        # ---------- phase 3: output projection (partial over this core's
        # 128 channels; host sums the 8 cores) ----------
        for b in range(B):
            for ti in range(T // 128):
                for fb in range(C // TB):
                    ps = psA.tile([128, TB], F32, name="y_ps", tag="psA")
                    nc.tensor.matmul(
                        ps[:],
                        attnT[b][:, ti * 128 : (ti + 1) * 128],
                        wp_sb[:, fb * TB : (fb + 1) * TB],
                        start=True,
                        stop=True,
                    )
                    ysb = ypool.tile([128, TB], F32, name="ysb", tag="ysb")
                    nc.any.tensor_copy(out=ysb[:], in_=ps[:])
                    nc.sync.dma_start(
                        y[b * T + ti * 128 : b * T + (ti + 1) * 128,
                          fb * TB : (fb + 1) * TB],
                        ysb[:],
                    )
    nc.compile()
    return nc


def make_in_maps(x, w_attn, w_proj):
    """Host-side sharding into the per-core layouts."""
    x = np.asarray(x, dtype=np.float32)
    w_attn = np.asarray(w_attn, dtype=np.float32)
    w_proj = np.asarray(w_proj, dtype=np.float32)

    xT = np.ascontiguousarray(x.reshape(BT, C).T)           # [1024, 4096]
    wpT_full = np.ascontiguousarray(w_proj.T)               # [c_in, f_out]

    in_maps = []
    for c in range(NCORES):
        rows = []
        for sec in range(3):                                # q, k, v
            for h in (HPC * c, HPC * c + 1):
                rows.extend(range(sec * C + h * D, sec * C + (h + 1) * D))
        wqkvT = np.ascontiguousarray(w_attn[rows, :].T)     # [1024, 384]
        wpT = np.ascontiguousarray(
            wpT_full[c * HPC * D : (c + 1) * HPC * D, :]    # [128, 1024]
        )
        consts = np.stack(
            [
                np.eye(128, dtype=np.float32),
                np.tril(np.ones((128, 128), np.float32)).T,  # keep kt <= qt
            ]
        )
        in_maps.append({"xT": xT, "wqkvT": wqkvT, "wpT": wpT, "consts": consts})
    return in_maps


_PROGRAM = None


def _program():
    global _PROGRAM
    if _PROGRAM is None:
        _PROGRAM = build_program()
    return _PROGRAM


def kernel(x, w_attn, w_proj):
    from concourse.bass_utils import run_bass_kernel_spmd

    res = run_bass_kernel_spmd(
        _program(), make_in_maps(x, w_attn, w_proj), list(range(NCORES))
    )
    out = res.results[0]["y"].astype(np.float32, copy=True)
    for i in range(1, NCORES):
        out += res.results[i]["y"]
    return out.reshape(B, T, C)


# revision 21
# speedup vs baseline: 1.0057x; 1.0057x over previous
"""Causal self-attention on 8 Trainium2 NeuronCores.

Sharding: 2 heads per core (tensor parallel).  The host pre-transposes the
activations/weights into the layouts the PE array wants, each core computes
QKV -> causal attention -> its partial of the output projection for its two
heads, and the host sums the 8 partial projections (row-parallel linear).

Per-core device program (SPMD, different data per core):
  xT    [1024, 4096]  x transposed, rows=embed c, cols=token t (t = b*2048+tt)
  wqkvT [1024, 384]   w_attn rows for this core's heads, transposed.
                      f = [q_h0 d0..63 | q_h1 | k_h0 | k_h1 | v_h0 | v_h1]
  wpT   [128, 1024]   w_proj columns for this core's channels, transposed
  y     [4096, 1024]  partial output (sum over cores = final)

Dataflow (everything "transposed" so the PE contraction dim is the partition
dim with no on-device transposes of activations):
  qkvT[f, t]   = wqkvT_tile.T @ xT_tile            (accumulate over 8 c-tiles)
  S^T[kt, qt]  = kT_tile.T @ qT_block              (K = head dim 64)
  P^T          = exp(S^T / 32)                     (ACT; no max subtraction --
                                                    scores are O(1), exp safe)
  causal mask  = multiply diagonal 128x128 block by 0/1 lower-tri tile
  outT[65,qt] += [V | ones].T @ P^T                (row 64 = softmax sums)
  attnT        = outT[0:64] * (1 / outT[64])       (broadcast along partitions)
  y[t, f]      = attnT_tile.T @ wpT                (partial; host sums cores)

All matmuls run as float32r (fp32 bitcast): 1 PE cycle/row when the moving
free dim is >= 256 -- full bf16-class speed with ~fp22 mantissa precision.
"""

import numpy as np

B, T, C = 2, 2048, 1024
H, D = 16, 64
NCORES = 8
HPC = H // NCORES          # heads per core = 2
BT = B * T                 # 4096 tokens total
TB = 512                   # token block (matmul moving free dim)
CK = C // 128              # 8 contraction tiles for the projections
NTB = BT // TB             # 8 token blocks
NQB = T // TB              # 4 q blocks per batch
NKT = T // 128             # 16 kt tiles per batch
SCALE = 1.0 / 32.0         # 1 / sqrt(C)


def build_program():
    """Build the single-core Bass program (same program runs on all 8 cores)."""
    from contextlib import ExitStack

    import concourse.mybir as mybir
    import concourse.tile as tile
    from concourse import bacc, library_config

    dt = mybir.dt
    F32 = dt.float32
    F32R = dt.float32r

    nc = bacc.Bacc("TRN2")
    xT = nc.dram_tensor("xT", [C, BT], F32, kind="ExternalInput").ap()
    wqkvT = nc.dram_tensor("wqkvT", [C, 3 * HPC * D], F32, kind="ExternalInput").ap()
    wpT = nc.dram_tensor("wpT", [HPC * D, C], F32, kind="ExternalInput").ap()
    # consts[0] = 128x128 identity, consts[1] = causal keep-mask
    # (mask[kt, qt] = 1.0 where kt <= qt)
    consts = nc.dram_tensor("consts", [2, 128, 128], F32, kind="ExternalInput").ap()
    y = nc.dram_tensor("y", [BT, C], F32, kind="ExternalOutput").ap()

    with ExitStack() as ctx:
        tc = ctx.enter_context(tile.TileContext(nc))
        const = ctx.enter_context(tc.tile_pool(name="const", bufs=1))
        xpool = ctx.enter_context(tc.tile_pool(name="xload", bufs=4))
        ppool = ctx.enter_context(tc.tile_pool(name="pexp", bufs=6))
        npool = ctx.enter_context(tc.tile_pool(name="norm", bufs=2))
        ypool = ctx.enter_context(tc.tile_pool(name="yout", bufs=3))
        psA = ctx.enter_context(tc.tile_pool(name="psA", bufs=4, space="PSUM"))
        psPV = ctx.enter_context(tc.tile_pool(name="psPV", bufs=2, space="PSUM"))

        # ---------- constants / persistent SBUF ----------
        w_sb = const.tile([128, CK, 3 * HPC * D], F32R, name="w_sb")
        nc.sync.dma_start(w_sb[:], wqkvT.rearrange("(a p) f -> p a f", p=128).bitcast(F32R))
        wp_sb = const.tile([128, C], F32R, name="wp_sb")
        nc.sync.dma_start(wp_sb[:], wpT.bitcast(F32R))

        ident = const.tile([128, 128], F32R, name="ident")
        nc.sync.dma_start(ident[:], consts[0].bitcast(F32R))
        trimask = const.tile([128, 128], F32, name="trimask")
        nc.sync.dma_start(trimask[:], consts[1])
        # partition_broadcast lives in the "attn" GPSIMD library; same-engine
        # FIFO order guarantees this lands before the broadcasts.
        nc.gpsimd.load_library(library_config.attn)

        # Per-batch transposed activations, heads packed on partitions
        # (h0 -> partitions 0:64, h1 -> 64:128).
        qT = [const.tile([128, T], F32R, name=f"qT{b}") for b in range(B)]
        kT = [const.tile([128, T], F32R, name=f"kT{b}") for b in range(B)]
        vT = [const.tile([128, T], F32R, name=f"vT{b}") for b in range(B)]
        attnT = [const.tile([128, T], F32R, name=f"attnT{b}") for b in range(B)]

        # [V | ones] stationary tiles for PV: V1[:, b, h, kti, 0:64] = V natural
        # [kt, d]; column 64 = 1.0 so PV row 64 accumulates the softmax sums.
        V1 = const.tile([128, B, HPC, NKT, 65], F32R, name="V1")
        nc.vector.memset(V1[:, :, :, :, 64:65].bitcast(F32), 1.0)

        # ---------- phase 1: QKV projection ----------
        dest = {0: qT, 1: kT, 2: vT}
        for tb in range(NTB):
            b, tcol = divmod(tb, NTB // B)
            xts = []
            for ci in range(CK):
                xt = xpool.tile([128, TB], F32R, name="xt", tag="xt")
                nc.sync.dma_start(
                    xt[:],
                    xT[ci * 128 : (ci + 1) * 128, tb * TB : (tb + 1) * TB].bitcast(F32R),
                )
                xts.append(xt)
            for fi in range(3):
                ps = psA.tile([128, TB], F32, name="qkv_ps", tag="psA")
                for ci in range(CK):
                    nc.tensor.matmul(
                        ps[:],
                        w_sb[:, ci, fi * 128 : (fi + 1) * 128],
                        xts[ci][:],
                        start=(ci == 0),
                        stop=(ci == CK - 1),
                    )
                nc.vector.tensor_copy(
                    out=dest[fi][b][:, tcol * TB : (tcol + 1) * TB], in_=ps[:]
                )

            # As soon as a batch's vT is complete, build its V-natural tiles
            # (PE transpose of 64-row slices through the identity).
            if tcol == NTB // B - 1:
                for h in range(HPC):
                    hp = slice(h * 64, (h + 1) * 64)
                    for kti in range(NKT):
                        tr = psA.tile([128, 64], F32R, name="vtr", tag="psA")
                        nc.tensor.transpose(
                            tr[:], vT[b][hp, kti * 128 : (kti + 1) * 128], ident[hp, hp]
                        )
                        nc.vector.tensor_copy(out=V1[:, b, h, kti, 0:64], in_=tr[:])

        # ---------- phase 2: causal attention ----------
        # Both heads interleaved per (b, qb) and PV software-pipelined one kt
        # tile behind the scores so the PE never stalls on the ACT exp.
        # Unnormalized [PV | sums] results are copied to SBUF (freeing PSUM)
        # and all 16 sum-rows are collected so one batched reciprocal covers
        # the whole kernel (a [1, N] DVE reciprocal is ~3.4 us — single lane).
        pvs = const.tile([65, B, HPC, NQB, TB], F32, name="pvs")
        sums = const.tile([B * HPC * NQB, TB], F32, name="sums")
        recip = const.tile([B * HPC * NQB, TB], F32, name="recip")

        def sum_row(b, h, qb):
            return b * HPC * NQB + h * NQB + qb

        for b in range(B):
            for qb in range(NQB):
                nkt = 4 * qb + 4
                pv = [
                    psPV.tile([65, TB], F32, name=f"pv_ps{h}", tag=f"psPV{h}")
                    for h in range(HPC)
                ]
                stages = []  # deferred PV matmuls, one kti behind the scores

                def flush(n=None):
                    while stages and (n is None or len(stages) > n):
                        stages.pop(0)()

                for kti in range(nkt):
                    qs = max(0, kti * 128 - qb * TB)  # local col start
                    N = TB - qs
                    Ps = []
                    for h in range(HPC):
                        hp = slice(h * 64, (h + 1) * 64)
                        sps = psA.tile([128, TB], F32, name="s_ps", tag="psA")
                        nc.tensor.matmul(
                            sps[:, 0:N],
                            kT[b][hp, kti * 128 : (kti + 1) * 128],
                            qT[b][hp, qb * TB + qs : (qb + 1) * TB],
                            start=True,
                            stop=True,
                        )
                        P = ppool.tile([128, TB], F32R, name="Pt", tag="P")
                        nc.scalar.activation(
                            P[:, 0:N],
                            sps[:, 0:N],
                            mybir.ActivationFunctionType.Exp,
                            scale=SCALE,
                        )
                        if kti * 128 >= qb * TB:
                            # diagonal tile: first 128 cols hold the triangle
                            nc.vector.tensor_mul(P[:, 0:128], P[:, 0:128], trimask[:])
                        Ps.append(P)

                    def pv_step(kti=kti, qs=qs, N=N, Ps=Ps):
                        for h in range(HPC):
                            nc.tensor.matmul(
                                pv[h][:, qs:TB],
                                V1[:, b, h, kti, :],
                                Ps[h][:, 0:N],
                                start=(kti == 0),
                                stop=(kti == nkt - 1),
                            )

                    stages.append(pv_step)
                    flush(1)
                flush()

                for h in range(HPC):
                    nc.vector.tensor_copy(out=pvs[:, b, h, qb, :], in_=pv[h][:])
                    # SBUF->SBUF DMA: engines need 32-aligned partition bases,
                    # DMA can scatter a row onto any partition.
                    nc.sync.dma_start(
                        sums[sum_row(b, h, qb) : sum_row(b, h, qb) + 1, :],
                        pvs[64:65, b, h, qb, :],
                    )

        # one batched reciprocal for every (b, h, qb) sum row
        nc.vector.reciprocal(recip[:], sums[:])

        # ---------- phase 3: normalize + output projection ----------
        for b in range(B):
            for qb in range(NQB):
                for h in range(HPC):
                    hp = slice(h * 64, (h + 1) * 64)
                    rt = npool.tile([1, TB], F32, name="rt", tag="rt")
                    nc.sync.dma_start(
                        rt[:], recip[sum_row(b, h, qb) : sum_row(b, h, qb) + 1, :]
                    )
                    bc = npool.tile([64, TB], F32, name="bc", tag="bc")
                    nc.gpsimd.partition_broadcast(bc[:], rt[:])
                    nc.vector.tensor_mul(
                        attnT[b][hp, qb * TB : (qb + 1) * TB],
                        pvs[0:64, b, h, qb, :],
                        bc[:],
                    )
                for ti in range(4 * qb, 4 * qb + 4):
                    for fb in range(C // TB):
                        ps = psA.tile([128, TB], F32, name="y_ps", tag="psA")
                        nc.tensor.matmul(
                            ps[:],
                            attnT[b][:, ti * 128 : (ti + 1) * 128],
                            wp_sb[:, fb * TB : (fb + 1) * TB],
                            start=True,
                            stop=True,
                        )
                        ysb = ypool.tile([128, TB], F32, name="ysb", tag="ysb")
                        nc.vector.tensor_copy(out=ysb[:], in_=ps[:])
                        nc.sync.dma_start(
                            y[b * T + ti * 128 : b * T + (ti + 1) * 128,
                              fb * TB : (fb + 1) * TB],
                            ysb[:],
                        )
    nc.compile()
    return nc


def make_in_maps(x, w_attn, w_proj):
    """Host-side sharding into the per-core layouts."""
    x = np.asarray(x, dtype=np.float32)
    w_attn = np.asarray(w_attn, dtype=np.float32)
    w_proj = np.asarray(w_proj, dtype=np.float32)

    xT = np.ascontiguousarray(x.reshape(BT, C).T)           # [1024, 4096]
    wpT_full = np.ascontiguousarray(w_proj.T)               # [c_in, f_out]

    in_maps = []
    for c in range(NCORES):
        rows = []
        for sec in range(3):                                # q, k, v
            for h in (HPC * c, HPC * c + 1):
                rows.extend(range(sec * C + h * D, sec * C + (h + 1) * D))
        wqkvT = np.ascontiguousarray(w_attn[rows, :].T)     # [1024, 384]
        wpT = np.ascontiguousarray(
            wpT_full[c * HPC * D : (c + 1) * HPC * D, :]    # [128, 1024]
        )
        consts = np.stack(
            [
                np.eye(128, dtype=np.float32),
                np.tril(np.ones((128, 128), np.float32)).T,  # keep kt <= qt
            ]
        )
        in_maps.append({"xT": xT, "wqkvT": wqkvT, "wpT": wpT, "consts": consts})
    return in_maps


_PROGRAM = None


def _program():
    global _PROGRAM
    if _PROGRAM is None:
        _PROGRAM = build_program()
    return _PROGRAM


def kernel(x, w_attn, w_proj):
    from concourse.bass_utils import run_bass_kernel_spmd

    res = run_bass_kernel_spmd(
        _program(), make_in_maps(x, w_attn, w_proj), list(range(NCORES))
    )
    out = res.results[0]["y"].astype(np.float32, copy=True)
    for i in range(1, NCORES):
        out += res.results[i]["y"]
    return out.reshape(B, T, C)


# revision 22
# speedup vs baseline: 1.2717x; 1.2645x over previous
"""Causal self-attention on 8 Trainium2 NeuronCores.

Sharding: 2 heads per core (tensor parallel).  The host pre-transposes the
activations/weights into the layouts the PE array wants, each core computes
QKV -> causal attention -> its partial of the output projection for its two
heads, and the host sums the 8 partial projections (row-parallel linear).

Per-core device program (SPMD, different data per core):
  xT    [1024, 4096]  x transposed, rows=embed c, cols=token t (t = b*2048+tt)
  wqkvT [1024, 384]   w_attn rows for this core's heads, transposed.
                      f = [q_h0 d0..63 | q_h1 | k_h0 | k_h1 | v_h0 | v_h1]
  wpT   [128, 1024]   w_proj columns for this core's channels, transposed
  y     [4096, 1024]  partial output (sum over cores = final)

Dataflow (everything "transposed" so the PE contraction dim is the partition
dim with no on-device transposes of activations):
  qkvT[f, t]   = wqkvT_tile.T @ xT_tile            (accumulate over 8 c-tiles)
  S^T[kt, qt]  = kT_tile.T @ qT_block              (K = head dim 64)
  P^T          = exp(S^T / 32)                     (ACT; no max subtraction --
                                                    scores are O(1), exp safe)
  causal mask  = multiply diagonal 128x128 block by 0/1 lower-tri tile
  outT[65,qt] += [V | ones].T @ P^T                (row 64 = softmax sums)
  attnT        = outT[0:64] * (1 / outT[64])       (broadcast along partitions)
  y[t, f]      = attnT_tile.T @ wpT                (partial; host sums cores)

All matmuls run as float32r (fp32 bitcast): 1 PE cycle/row when the moving
free dim is >= 256 -- full bf16-class speed with ~fp22 mantissa precision.
"""

import numpy as np

B, T, C = 2, 2048, 1024
H, D = 16, 64
NCORES = 8
HPC = H // NCORES          # heads per core = 2
BT = B * T                 # 4096 tokens total
TB = 512                   # token block (matmul moving free dim)
CK = C // 128              # 8 contraction tiles for the projections
NTB = BT // TB             # 8 token blocks
NQB = T // TB              # 4 q blocks per batch
NKT = T // 128             # 16 kt tiles per batch
SCALE = 1.0 / 32.0         # 1 / sqrt(C)


def build_program():
    """Build the single-core Bass program (same program runs on all 8 cores)."""
    from contextlib import ExitStack

    import concourse.mybir as mybir
    import concourse.tile as tile
    from concourse import bacc, library_config

    dt = mybir.dt
    F32 = dt.float32
    F32R = dt.float32r

    nc = bacc.Bacc("TRN2")
    xT = nc.dram_tensor("xT", [C, BT], F32, kind="ExternalInput").ap()
    wqkvT = nc.dram_tensor("wqkvT", [C, 3 * HPC * D], F32, kind="ExternalInput").ap()
    wpT = nc.dram_tensor("wpT", [HPC * D, C], F32, kind="ExternalInput").ap()
    # consts[0] = 128x128 identity, consts[1] = causal keep-mask
    # (mask[kt, qt] = 1.0 where kt <= qt)
    consts = nc.dram_tensor("consts", [2, 128, 128], F32, kind="ExternalInput").ap()
    y = nc.dram_tensor("y", [BT, C], F32, kind="ExternalOutput").ap()

    with ExitStack() as ctx:
        tc = ctx.enter_context(tile.TileContext(nc))
        const = ctx.enter_context(tc.tile_pool(name="const", bufs=1))
        xpool = ctx.enter_context(tc.tile_pool(name="xload", bufs=12))
        ppool = ctx.enter_context(tc.tile_pool(name="pexp", bufs=4))
        npool = ctx.enter_context(tc.tile_pool(name="norm", bufs=2))
        ypool = ctx.enter_context(tc.tile_pool(name="yout", bufs=3))
        psA = ctx.enter_context(tc.tile_pool(name="psA", bufs=2, space="PSUM"))
        psPV = ctx.enter_context(tc.tile_pool(name="psPV", bufs=2, space="PSUM"))

        # ---------- constants / persistent SBUF ----------
        w_sb = const.tile([128, CK, 3 * HPC * D], F32R, name="w_sb")
        nc.sync.dma_start(w_sb[:], wqkvT.rearrange("(a p) f -> p a f", p=128).bitcast(F32R))
        wp_sb = const.tile([128, C], F32R, name="wp_sb")
        nc.sync.dma_start(wp_sb[:], wpT.bitcast(F32R))

        ident = const.tile([128, 128], F32R, name="ident")
        nc.sync.dma_start(ident[:], consts[0].bitcast(F32R))
        trimask2 = const.tile([128, HPC, 128], F32, name="trimask2")
        for _h in range(HPC):
            nc.sync.dma_start(trimask2[:, _h, :], consts[1])
        # partition_broadcast lives in the "attn" GPSIMD library; same-engine
        # FIFO order guarantees this lands before the broadcasts.
        nc.gpsimd.load_library(library_config.attn)

        # Per-batch transposed activations, heads packed on partitions
        # (h0 -> partitions 0:64, h1 -> 64:128).
        qT = [const.tile([128, T], F32R, name=f"qT{b}") for b in range(B)]
        kT = [const.tile([128, T], F32R, name=f"kT{b}") for b in range(B)]
        vT = [const.tile([128, T], F32R, name=f"vT{b}") for b in range(B)]
        attnT = [const.tile([128, T], F32R, name=f"attnT{b}") for b in range(B)]

        # [V | ones] stationary tiles for PV: V1[:, b, h, kti, 0:64] = V natural
        # [kt, d]; column 64 = 1.0 so PV row 64 accumulates the softmax sums.
        V1 = const.tile([128, B, HPC, NKT, 65], F32R, name="V1")
        nc.vector.memset(V1[:, :, :, :, 64:65].bitcast(F32), 1.0)

        # ---------- phase 1: QKV projection ----------
        dest = {0: qT, 1: kT, 2: vT}
        for tb in range(NTB):
            b, tcol = divmod(tb, NTB // B)
            xts = []
            for ci in range(CK):
                xt = xpool.tile([128, TB], F32R, name="xt", tag="xt")
                nc.sync.dma_start(
                    xt[:],
                    xT[ci * 128 : (ci + 1) * 128, tb * TB : (tb + 1) * TB].bitcast(F32R),
                )
                xts.append(xt)
            for fi in range(3):
                ps = psA.tile([128, TB], F32, name="qkv_ps", tag="psA")
                for ci in range(CK):
                    nc.tensor.matmul(
                        ps[:],
                        w_sb[:, ci, fi * 128 : (fi + 1) * 128],
                        xts[ci][:],
                        start=(ci == 0),
                        stop=(ci == CK - 1),
                    )
                nc.scalar.copy(
                    out=dest[fi][b][:, tcol * TB : (tcol + 1) * TB], in_=ps[:]
                )

            # As soon as a batch's vT is complete, build its V-natural tiles
            # (PE transpose of 64-row slices through the identity).
            if tcol == NTB // B - 1:
                for h in range(HPC):
                    hp = slice(h * 64, (h + 1) * 64)
                    for kti in range(NKT):
                        tr = psA.tile([128, 64], F32R, name="vtr", tag="psA")
                        nc.tensor.transpose(
                            tr[:], vT[b][hp, kti * 128 : (kti + 1) * 128], ident[hp, hp]
                        )
                        nc.vector.tensor_copy(out=V1[:, b, h, kti, 0:64], in_=tr[:])

        # ---------- phase 2: causal attention ----------
        # Both heads interleaved per (b, qb) and PV software-pipelined one kt
        # tile behind the scores so the PE never stalls on the ACT exp.
        # Unnormalized [PV | sums] results are copied to SBUF (freeing PSUM)
        # and all 16 sum-rows are collected so one batched reciprocal covers
        # the whole kernel (a [1, N] DVE reciprocal is ~3.4 us — single lane).
        pvs = const.tile([65, B, HPC, NQB, TB], F32, name="pvs")
        sums = const.tile([B * HPC * NQB, TB], F32, name="sums")
        recip = const.tile([B * HPC * NQB, TB], F32, name="recip")

        def sum_row(b, h, qb):
            return b * HPC * NQB + h * NQB + qb

        for b in range(B):
            for qb in range(NQB):
                nkt = 4 * qb + 4
                pv = [
                    psPV.tile([65, TB], F32, name=f"pv_ps{h}", tag=f"psPV{h}")
                    for h in range(HPC)
                ]
                stages = []  # deferred PV matmuls, one kti behind the scores

                def flush(n=None):
                    while stages and (n is None or len(stages) > n):
                        stages.pop(0)()

                for kti in range(nkt):
                    qs = max(0, kti * 128 - qb * TB)  # local col start
                    N = TB - qs
                    # both heads' scores in one 2-bank PSUM tile -> one exp
                    sps = psA.tile([128, HPC, TB], F32, name="s_ps", tag="psA")
                    for h in range(HPC):
                        hp = slice(h * 64, (h + 1) * 64)
                        nc.tensor.matmul(
                            sps[:, h, 0:N],
                            kT[b][hp, kti * 128 : (kti + 1) * 128],
                            qT[b][hp, qb * TB + qs : (qb + 1) * TB],
                            start=True,
                            stop=True,
                        )
                    P = ppool.tile([128, HPC, TB], F32R, name="Pt", tag="P")
                    nc.scalar.activation(
                        P[:, :, 0:N],
                        sps[:, :, 0:N],
                        mybir.ActivationFunctionType.Exp,
                        scale=SCALE,
                    )
                    if kti * 128 >= qb * TB:
                        # diagonal tile: first 128 cols of each head hold the
                        # triangle; one DVE mult covers both heads
                        nc.vector.tensor_mul(
                            P[:, :, 0:128], P[:, :, 0:128], trimask2[:]
                        )

                    def pv_step(kti=kti, qs=qs, N=N, P=P):
                        for h in range(HPC):
                            nc.tensor.matmul(
                                pv[h][:, qs:TB],
                                V1[:, b, h, kti, :],
                                P[:, h, 0:N],
                                start=(kti == 0),
                                stop=(kti == nkt - 1),
                            )

                    stages.append(pv_step)
                    flush(1)
                flush()

                for h in range(HPC):
                    nc.vector.tensor_copy(out=pvs[:, b, h, qb, :], in_=pv[h][:])
                    # SBUF->SBUF DMA: engines need 32-aligned partition bases,
                    # DMA can scatter a row onto any partition.
                    nc.sync.dma_start(
                        sums[sum_row(b, h, qb) : sum_row(b, h, qb) + 1, :],
                        pvs[64:65, b, h, qb, :],
                    )

        # one batched reciprocal for every (b, h, qb) sum row
        nc.vector.reciprocal(recip[:], sums[:])

        # ---------- phase 3: normalize, then output projection ----------
        for b in range(B):
            for qb in range(NQB):
                for h in range(HPC):
                    hp = slice(h * 64, (h + 1) * 64)
                    rt = npool.tile([1, TB], F32, name="rt", tag="rt")
                    nc.sync.dma_start(
                        rt[:], recip[sum_row(b, h, qb) : sum_row(b, h, qb) + 1, :]
                    )
                    bc = npool.tile([64, TB], F32, name="bc", tag="bc")
                    nc.gpsimd.partition_broadcast(bc[:], rt[:])
                    nc.vector.tensor_mul(
                        attnT[b][hp, qb * TB : (qb + 1) * TB],
                        pvs[0:64, b, h, qb, :],
                        bc[:],
                    )
        for b in range(B):
            for ti in range(T // 128):
                for fb in range(C // TB):
                    ps = psA.tile([128, TB], F32, name="y_ps", tag="psA")
                    nc.tensor.matmul(
                        ps[:],
                        attnT[b][:, ti * 128 : (ti + 1) * 128],
                        wp_sb[:, fb * TB : (fb + 1) * TB],
                        start=True,
                        stop=True,
                    )
                    ysb = ypool.tile([128, TB], F32, name="ysb", tag="ysb")
                    if (ti + fb) % 2 == 0:
                        nc.vector.tensor_copy(out=ysb[:], in_=ps[:])
                    else:
                        nc.scalar.copy(out=ysb[:], in_=ps[:])
                    nc.sync.dma_start(
                        y[b * T + ti * 128 : b * T + (ti + 1) * 128,
                          fb * TB : (fb + 1) * TB],
                        ysb[:],
                    )
    nc.compile()
    return nc


def make_in_maps(x, w_attn, w_proj):
    """Host-side sharding into the per-core layouts."""
    x = np.asarray(x, dtype=np.float32)
    w_attn = np.asarray(w_attn, dtype=np.float32)
    w_proj = np.asarray(w_proj, dtype=np.float32)

    xT = np.ascontiguousarray(x.reshape(BT, C).T)           # [1024, 4096]
    wpT_full = np.ascontiguousarray(w_proj.T)               # [c_in, f_out]

    in_maps = []
    for c in range(NCORES):
        rows = []
        for sec in range(3):                                # q, k, v
            for h in (HPC * c, HPC * c + 1):
                rows.extend(range(sec * C + h * D, sec * C + (h + 1) * D))
        wqkvT = np.ascontiguousarray(w_attn[rows, :].T)     # [1024, 384]
        wpT = np.ascontiguousarray(
            wpT_full[c * HPC * D : (c + 1) * HPC * D, :]    # [128, 1024]
        )
        consts = np.stack(
            [
                np.eye(128, dtype=np.float32),
                np.tril(np.ones((128, 128), np.float32)).T,  # keep kt <= qt
            ]
        )
        in_maps.append({"xT": xT, "wqkvT": wqkvT, "wpT": wpT, "consts": consts})
    return in_maps


_PROGRAM = None


def _program():
    global _PROGRAM
    if _PROGRAM is None:
        _PROGRAM = build_program()
    return _PROGRAM


def kernel(x, w_attn, w_proj):
    from concourse.bass_utils import run_bass_kernel_spmd

    res = run_bass_kernel_spmd(
        _program(), make_in_maps(x, w_attn, w_proj), list(range(NCORES))
    )
    out = res.results[0]["y"].astype(np.float32, copy=True)
    for i in range(1, NCORES):
        out += res.results[i]["y"]
    return out.reshape(B, T, C)
